# revision 80
# baseline (speedup 1.0000x reference)
"""Trainium2 Bass kernel for nn_Attention_19739669692939 (sparse_attention).

Reference computation (shapes: L=1024, B=64, C=1024, D=512, E=512):
    Wa_e = W_attn[:, :C]        # [E, C]
    Wa_s = W_attn[:, C:]        # [E, D]
    pre  = enc_output @ Wa_e.T + s @ Wa_s.T     # [L, B, E] (s broadcast over L)
    engry = tanh(pre)
    att[b, l] = engry[l, b, :] @ W_v[0, :]
    out = softmax(att, axis=-1)                 # [B, 1024]

Distribution: pure data-parallel over batch. Core i handles batches
[8i, 8i+8); no collectives.

Per core: a 8192x1024 @ 1024x512 matmul in bf16 on the PE, fused bias+tanh
on ACT, and the W_v reduction on the DVE (stream-transpose + grouped
reduce), then a fragment-layout softmax.

The PE contracts over partitions, so enc needs its feature dim (c) on
partitions: enc is cast f32->bf16 during the HBM load (free, SWDGE cast
path), then transposed on the PE ([128,128] is_transpose matmuls against
identity; bf16 halves the weight-load cost vs f32). f32->bf16 keeps rel
err ~2e-3, well under the 2e-2 gate.

Schedule notes (measured on HW, 190.0us vs 193.5us for the previous
all-PE version):
- The main matmul (512 N=512 bf16 MMs) runs at the stream-rate roofline
  (~215 ns each, ~110 us/core = the bf16 compute roofline for 8.6 GFLOP);
  PE transposes pipeline at ~56-90ns behind them.
- The W_v reduction used to be 256 masked PE matmuls (55us of extra
  engry streams). It now runs on the DVE: per (lc, b), a per-partition
  wv multiply (tensor_scalar, wv is per-e), one [128, 2048] 32x32
  stream-transpose, and one grouped XY-reduce producing e-block partials
  attS2[32i+a, b, lc, j] for l = 32j+a. The four partition-block
  partials are folded once in the epilogue (TensorTensor requires equal
  SBUF base partitions, so slices are staged through copies).
- encT PSUM->SBUF copies go 1:3 to DVE:ACT — the DVE carries the wv
  chain (~120us) and must stay under the PE span (~163us).
- Softmax runs in the fragment layout: exp on [32, 256], column sums via
  two [32,128]-stationary PE matmuls, per-b totals by grouped reduce,
  reciprocal, a rank-1 ones matmul to broadcast the scales back across
  partitions, and two PE transposes to emit [b, l]-contiguous rows.
- SWDGE ring order (s, W, first enc chunks) is chosen so the PE's data
  dependencies resolve in issue order during the ramp; gpsimd carries
  ONLY SWDGE work (its cores process the descriptor ring; any compute
  placed there throttles the loads).
"""

import numpy as np

import concourse.bass as bass
import concourse.mybir as mybir
from concourse import bacc
from concourse.bass_utils import run_bass_kernel_spmd
from concourse.masks import make_identity
from concourse.tile import TileContext

F32 = mybir.dt.float32
BF16 = mybir.dt.bfloat16
AF = mybir.ActivationFunctionType

L = 1024          # enc length
B = 64            # global batch
BL = 8            # batch per core
C = 1024          # enc feature dim (2*enc_hid)
D = 512           # dec feature dim
E = 512           # engry dim
NCORES = 8

NCB = C // 128    # 8 c-blocks
NDB = D // 128    # 4 d-blocks
NEB = E // 128    # 4 e-blocks
LCH = 512         # l-chunk processed per inner iteration
NLC = L // LCH    # 2 chunks
KSUB = LCH // 128  # 4 l-subblocks per chunk

NWB = (C + D) // 128  # 12 blocks over W_attn's column (c/d) axis


def build_nc():
    nc = bacc.Bacc("TRN2", target_bir_lowering=False, debug=False)

    enc = nc.dram_tensor("enc_output", [L, BL, C], F32, kind="ExternalInput").ap()
    s = nc.dram_tensor("s", [1, BL, D], F32, kind="ExternalInput").ap()
    w_attn = nc.dram_tensor("W_attn", [E, C + D], F32, kind="ExternalInput").ap()
    w_v = nc.dram_tensor("W_v", [1, E], F32, kind="ExternalInput").ap()
    out = nc.dram_tensor("out", [BL, L], F32, kind="ExternalOutput").ap()

    with TileContext(nc) as tc:
        with (
            tc.tile_pool(name="consts", bufs=1) as consts,
            tc.tile_pool(name="nat", bufs=8) as nat_pool,
            tc.tile_pool(name="encT", bufs=4) as encT_pool,
            tc.tile_pool(name="engry", bufs=2) as engry_pool,
            tc.tile_pool(name="prod", bufs=2) as prod_pool,
            tc.tile_pool(name="tmega", bufs=2) as tmega_pool,
            tc.tile_pool(name="sred", bufs=2) as s_pool,
            tc.tile_pool(name="tp", bufs=4, space="PSUM") as tp_pool,
            tc.tile_pool(name="pre", bufs=4, space="PSUM") as pre_pool,
        ):
            # ---------------- setup: constants and weights ----------------
            ident = consts.tile([128, 128], F32, tag="ident")
            make_identity(nc, ident)
            identB = consts.tile([128, 128], BF16, tag="identB")
            nc.vector.tensor_copy(identB[:], ident[:])

            # s first on the SWDGE ring (tiny), then W, then the first enc
            # chunks — the ring drains in order, so order = priority.
            s_sbuf = consts.tile([BL, D], BF16, tag="s_sbuf")
            nc.gpsimd.dma_start(out=s_sbuf[:], in_=s[0])

            # SWDGE ring order = completion order: the first enc chunk goes
            # before W so the PE's first transposes have data ASAP; W's four
            # chunks next (waT transposes start incrementally); then more enc.
            NPRE = 5
            pre_enc = {}

            def issue_enc_cast(j):
                lc0, b0 = divmod(j, BL)
                enc_t = nat_pool.tile([128, KSUB * C], BF16, tag="nat",
                                      name=f"enc_pre{j}")
                nc.gpsimd.dma_start(
                    out=enc_t.rearrange("p (k w c) -> p k w c", k=KSUB, w=NCB),
                    in_=enc[lc0 * LCH:lc0 * LCH + LCH, b0, :].rearrange(
                        "(k p) (w c) -> p k w c", p=128, w=NCB
                    ),
                )
                pre_enc[(lc0, b0)] = enc_t

            # W_attn: cast-load bf16 [e'(128 part), (r 4, w 12, cc 128)],
            # split per e-block so waT transposes can start early.
            wnat = consts.tile([128, NEB * (C + D)], BF16, tag="wnat")
            for r in range(NEB):
                nc.gpsimd.dma_start(
                    out=wnat[:, r * (C + D):(r + 1) * (C + D)].rearrange(
                        "p (w c) -> p w c", w=NWB),
                    in_=w_attn[r * 128:(r + 1) * 128, :].rearrange(
                        "p (w c) -> p w c", w=NWB),
                )

            for j in range(NPRE):
                issue_enc_cast(j)

            # Small HAM-warmup block: bridges the DMA-gated idle slots in
            # the setup phase so the clock-gate sees sustained activity.
            # (Never read; bacc has no instruction-level DCE.)
            warm_ps = tp_pool.tile([128, 512], BF16, tag="tp")
            for _ in range(28):
                nc.tensor.transpose(warm_ps[:, :128], identB[:], identB[:])

            # s -> sT [d(4x128 part), b(8)] — first PE work (s lands first)
            sT = consts.tile([128, NDB * BL], BF16, tag="sT")
            for db in range(NDB):
                tps = tp_pool.tile([128, 512], BF16, tag="tp")
                nc.tensor.transpose(
                    tps[:, :BL],
                    s_sbuf[:, db * 128:(db + 1) * 128],
                    identB[:BL, :BL],
                )
                nc.vector.tensor_copy(sT[:, db * BL:(db + 1) * BL], tps[:, :BL])

            # waT [cc(128 part), (w 12, e 512)] via PE transposes (bf16),
            # r-outer so each W chunk unlocks a dense 12-transpose burst —
            # the bursts chain into >3.4us of sustained PE work, releasing
            # the HAM clock-gate during setup instead of mid-main-loop.
            waT = consts.tile([128, NWB * E], BF16, tag="waT")
            for r in range(NEB):
                for w in range(NWB):
                    tpw = tp_pool.tile([128, 512], BF16, tag="tp",
                                       name=f"tpw_{r}_{w}")
                    nc.tensor.transpose(
                        tpw[:, :128],
                        wnat[:, r * (C + D) + w * 128: r * (C + D) + (w + 1) * 128],
                        identB[:],
                    )
                    dst = waT[:, w * E + r * 128: w * E + (r + 1) * 128]
                    if w % 2 == 0:
                        nc.vector.tensor_copy(dst, tpw[:, :128])
                    else:
                        nc.scalar.copy(dst, tpw[:, :128])

            # bias[e, b] = Wa_s @ s[b].T  — [e(4x128 part), b(8)] per e-block
            bias_sbuf = consts.tile([128, NEB * BL], F32, tag="bias")
            for eb in range(NEB):
                bps = tp_pool.tile([128, 512], F32, tag="tp")
                for db in range(NDB):
                    nc.tensor.matmul(
                        bps[:, :BL],
                        lhsT=waT[:, (NCB + db) * E + eb * 128:
                                 (NCB + db) * E + (eb + 1) * 128],
                        rhs=sT[:, db * BL:(db + 1) * BL],
                        start=(db == 0),
                        stop=(db == NDB - 1),
                    )
                nc.vector.tensor_copy(bias_sbuf[:, eb * BL:(eb + 1) * BL], bps[:, :BL])

            # W_v: [1, E] -> wvT [e(128 part), eb(4)] via f32 PE transposes.
            wv_sbuf = consts.tile([1, E], F32, tag="wv_sbuf")
            nc.sync.dma_start(out=wv_sbuf[:], in_=w_v[:])
            wvT = consts.tile([128, NEB], F32, tag="wvT")
            for eb in range(NEB):
                tpv = tp_pool.tile([128, 512], F32, tag="tp")
                nc.tensor.transpose(
                    tpv[:, :1],
                    wv_sbuf[:, eb * 128:(eb + 1) * 128],
                    ident[:1, :1],
                )
                nc.vector.tensor_copy(wvT[:, eb:eb + 1], tpv[:, :1])

            # epilogue constants for the DVE-based W_v reduction
            ones32r = consts.tile([1, 32], F32, tag="ones32r")
            nc.vector.memset(ones32r[:], 1.0)
            onescol_f = consts.tile([128, 1], F32, tag="onescol_f")
            nc.vector.memset(onescol_f[:], 1.0)
            # attS2[e-block-partials]: [128 (32i+a), (b 8, lc 2, j 16)] f32;
            # att[b, lc*512 + 32j + a] = sum_i (attS2a+attS2b)[32i+a, b, lc, j]
            attS2a = consts.tile([128, BL, NLC, 16], F32, tag="attS2a")

            # ---------------- main loop ----------------
            for lc in range(NLC):
                for b in range(BL):
                    l0 = lc * LCH
                    # enc chunk, cast f32->bf16 during DMA.
                    # layout: [p(128 l'), (k 4, cb 8, cc 128)]
                    if (lc, b) in pre_enc:
                        enc_t = pre_enc[(lc, b)]
                    else:
                        enc_t = nat_pool.tile([128, KSUB * C], BF16, tag="nat")
                        nc.gpsimd.dma_start(
                            out=enc_t.rearrange("p (k w c) -> p k w c",
                                                k=KSUB, w=NCB),
                            in_=enc[l0:l0 + LCH, b, :].rearrange(
                                "(k p) (w c) -> p k w c", p=128, w=NCB
                            ),
                        )
                    # PE transposes: two c-blocks share one full-bank PSUM
                    # tile (8 transposes, then a single [128,1024] copy).
                    # encT layout: [cc(128 part), (cb 8, l 512=k*128+l')]
                    encT = encT_pool.tile([128, NCB * LCH], BF16, tag="encT")
                    for cp in range(NCB // 2):
                        tpt = tp_pool.tile([128, 1024], BF16, tag="tp")
                        for half in range(2):
                            cb = 2 * cp + half
                            for k in range(KSUB):
                                nc.tensor.transpose(
                                    tpt[:, half * 512 + k * 128:
                                        half * 512 + (k + 1) * 128],
                                    enc_t[:, k * C + cb * 128:
                                          k * C + (cb + 1) * 128],
                                    identB[:],
                                )
                        if cp < 1:
                            nc.vector.tensor_copy(
                                encT[:, 2 * cp * LCH:(2 * cp + 2) * LCH], tpt[:])
                        else:
                            nc.scalar.copy(
                                encT[:, 2 * cp * LCH:(2 * cp + 2) * LCH], tpt[:])

                    engries = []
                    for eb in range(NEB):
                        pre = pre_pool.tile([128, LCH], F32, tag="pre")
                        for cb in range(NCB):
                            nc.tensor.matmul(
                                pre[:],
                                lhsT=waT[:, cb * E + eb * 128:
                                         cb * E + (eb + 1) * 128],
                                rhs=encT[:, cb * LCH:(cb + 1) * LCH],
                                start=(cb == 0),
                                stop=(cb == NCB - 1),
                            )
                        engry = engry_pool.tile([128, LCH], BF16, tag=f"engry{eb}",
                                                name=f"engry{eb}_{lc}_{b}")
                        nc.scalar.activation(
                            engry[:], pre[:], AF.Tanh,
                            bias=bias_sbuf[:, eb * BL + b: eb * BL + b + 1],
                            scale=1.0,
                        )
                        engries.append(engry)
                    # W_v reduction on the DVE (frees ~55us of PE streams):
                    # prod[e', (eb, l)] = engry * wv (per-partition scalar),
                    # stream-transpose 32x32 blocks, then grouped free-axis
                    # reduces turn the e-partition sum into per-block
                    # partials attS2[32i+a, b, lc, j] for l = 32j + a.
                    prod = prod_pool.tile([128, NEB * LCH], BF16, tag="prod",
                                          name=f"prod_{lc}_{b}")
                    for eb in range(NEB):
                        nc.vector.tensor_scalar_mul(
                            prod[:, eb * LCH:(eb + 1) * LCH],
                            engries[eb][:],
                            wvT[:, eb:eb + 1],
                        )
                    tm = tmega_pool.tile([128, NEB * LCH], BF16, tag="tm",
                                         name=f"tm_{lc}_{b}")
                    nc.vector.transpose(tm[:], prod[:])
                    nc.vector.reduce_sum(
                        attS2a[:, b, lc, :],
                        tm.rearrange("p (eb j b2) -> p j eb b2",
                                     eb=NEB, b2=32),
                        axis=mybir.AxisListType.XY)

            # ---------------- softmax epilogue ----------------
            # fold the 4 e-partition-blocks of attS2 (TensorTensor needs
            # equal SBUF base partitions, so stage via copies)
            c1 = consts.tile([64, BL * NLC * 16], F32, tag="c1")
            nc.vector.tensor_copy(c1[:], attS2a.rearrange(
                "p b l j -> p (b l j)")[64:128, :])
            a1 = consts.tile([64, BL * NLC * 16], F32, tag="a1")
            nc.vector.tensor_add(a1[:], attS2a.rearrange(
                "p b l j -> p (b l j)")[0:64, :], c1[:])
            c2 = consts.tile([32, BL * NLC * 16], F32, tag="c2")
            nc.vector.tensor_copy(c2[:], a1[32:64, :])
            attF = consts.tile([32, BL * NLC * 16], F32, tag="attF")
            nc.vector.tensor_add(attF[:], a1[0:32, :], c2[:])
            # exp (|logits| <= ||W_v||_1 ~ 18; skip max subtraction)
            attE = consts.tile([32, BL * NLC * 16], F32, tag="attE")
            nc.scalar.activation(attE[:], attF[:], AF.Exp)
            # column sums over the 32 'a' partitions, as two [32,128]
            # stationary matmuls; transpose to a row
            sumsRow = consts.tile([1, 256], F32, tag="sumsRow")
            for h in range(2):
                sp1 = tp_pool.tile([128, 512], F32, tag="tp",
                                   name=f"sp1_{h}")
                nc.tensor.matmul(sp1[:128, :1],
                                 lhsT=attE[:, h * 128:(h + 1) * 128],
                                 rhs=onescol_f[:32, :],
                                 start=True, stop=True)
                cs = consts.tile([128, 1], F32, tag="cs", name=f"cs{h}")
                nc.vector.tensor_copy(cs[:], sp1[:128, :1])
                sp2 = tp_pool.tile([128, 512], F32, tag="tp",
                                   name=f"sp2_{h}")
                nc.tensor.transpose(sp2[:1, :128], cs[:], ident[:])
                nc.vector.tensor_copy(sumsRow[:, h * 128:(h + 1) * 128],
                                      sp2[:1, :128])
            # per-b totals (cols are (b 8, lc 2, j 16)), reciprocal, expand
            tot = consts.tile([1, BL], F32, tag="tot")
            nc.vector.reduce_sum(
                tot[:], sumsRow.rearrange("p (b m) -> p b m", b=BL),
                axis=mybir.AxisListType.X)
            recipT = consts.tile([1, BL], F32, tag="recipT")
            nc.vector.reciprocal(recipT[:], tot[:])
            recipRow = consts.tile([1, 256], F32, tag="recipRow")
            for b in range(BL):
                nc.vector.tensor_scalar_mul(
                    recipRow[:, b * 32:(b + 1) * 32], ones32r[:],
                    recipT[:, b:b + 1])
            rrp = tp_pool.tile([128, 512], F32, tag="tp", name="rrp")
            nc.tensor.matmul(rrp[:32, :256], lhsT=ones32r[:],
                             rhs=recipRow[:], start=True, stop=True)
            nc.vector.tensor_mul(attE[:], attE[:], rrp[:32, :256])
            # transpose the normalized fragments to [(b, lc, j), a] rows and
            # store both halves with one copy + one DMA
            tpo = tp_pool.tile([128, 512], F32, tag="tp", name="tpo")
            for h in range(2):
                nc.tensor.transpose(tpo[:128, h * 32:(h + 1) * 32],
                                    attE[:, h * 128:(h + 1) * 128],
                                    ident[:32, :32])
            oco = consts.tile([128, 64], F32, tag="oco")
            nc.vector.tensor_copy(oco[:], tpo[:128, :64])
            nc.sync.dma_start(
                out=out.rearrange("b (l j a) -> (b l j) a", l=NLC, a=32)
                       .rearrange("(h r) a -> r h a", h=2),
                in_=oco.rearrange("p (h a) -> p h a", h=2),
            )

    nc.compile()
    return nc


_NC_CACHE = None


def _get_nc():
    global _NC_CACHE
    if _NC_CACHE is None:
        _NC_CACHE = build_nc()
    return _NC_CACHE


def make_in_maps(enc_output, s, W_attn, W_v):
    enc_output = np.asarray(enc_output, dtype=np.float32)
    s = np.asarray(s, dtype=np.float32)
    W_attn = np.ascontiguousarray(np.asarray(W_attn, dtype=np.float32))
    W_v = np.ascontiguousarray(np.asarray(W_v, dtype=np.float32))
    in_maps = []
    for i in range(NCORES):
        in_maps.append({
            "enc_output": np.ascontiguousarray(enc_output[:, i * BL:(i + 1) * BL, :]),
            "s": np.ascontiguousarray(s[:, i * BL:(i + 1) * BL, :]),
            "W_attn": W_attn,
            "W_v": W_v,
        })
    return in_maps


def kernel(enc_output, s, W_attn, W_v):
    nc = _get_nc()
    in_maps = make_in_maps(enc_output, s, W_attn, W_v)
    res = run_bass_kernel_spmd(nc, in_maps, core_ids=list(range(NCORES)))
    return np.concatenate([res.results[i]["out"] for i in range(NCORES)], axis=0)

